# revision 2
# baseline (speedup 1.0000x reference)
"""EnhancedTransformerBlock on 8 TRN2 NeuronCores — fp8 DoubleRow version.

Data-parallel over batch (1 element/core, no collectives). T-layout trunk
([feature, token], fp32 residuals) as in the bf16 baseline, but every large
GEMM runs fp8(e4m3) with perf_mode=DoubleRow: weights are host-quantized
with per-tensor power-of-2 scales into [P, K/256, 2, M] interleaved layout;
activations are quantized at eviction (fixed power-of-2 scales) into the
matching [P, K/256, 2, S] layout. Scores stay non-DR fp8 (K=64 per head,
head pairs packed onto PE row groups 0-1/2-3); exp is evicted straight to
fp8 with the quant scale folded into the activation bias (exp(x)*ES =
exp(x+ln ES)). Softmax rowsums ride the AV matmul as a 65th output row
(ones column in V); normalization is a K=1 broadcast matmul + one fused
DVE op writing fp8 ctx.

LN1/LN2 statistics and all residual arithmetic stay fp32 (xTf/accf slabs);
stats sums use fp8/bf16 ones-column matmuls as in the baseline.
"""
import sys

sys.path.insert(0, '/opt/trn_rl_repo')

import numpy as np
import ml_dtypes

import concourse.bass as bass
import concourse.bacc as bacc
import concourse.tile as tile
from concourse import mybir
from concourse.bass_utils import run_bass_kernel_spmd
from concourse.masks import make_identity

F32 = mybir.dt.float32
BF16 = mybir.dt.bfloat16
FP8 = mybir.dt.float8e4
AF = mybir.ActivationFunctionType
OP = mybir.AluOpType
DRM = mybir.MatmulPerfMode.DoubleRow

P = 128
B, S, H = 8, 1024, 1024
NH, HD = 16, 64
HF, HG = 4 * H, H // 2
HC = H // P          # 8 feature chunks
DC = H // 256        # 4 double-chunks (DoubleRow)
FC = HF // P         # 32 ffn chunks
GC = HG // P         # 4 gate chunks
QT = S // 512        # 2 q tiles of 512
EPS = 1e-5

# activation quant scales (powers of 2; fp8 is float so these only guard
# against overflow/subnormal-flush, they do not change relative error)
XS = 16.0    # x
QAS = 16.0   # Q/K rows
ES = 2.0     # exp(scores); fp8 overflow would need ~12-sigma score
VS = 32.0    # V
CS = 32.0    # ctx
X1S = 16.0   # x1 (pre-LN1, stats only)
YS = 16.0    # y1 (post-LN1)
RS = 32.0    # relu(gate1)

_BUILD_CACHE = {}


def _bcast_ap(param, n_part, n_free):
    ap = param[None, :]
    return bass.AP(tensor=ap.tensor, offset=ap.offset, ap=[[0, n_part], [1, n_free]])


def _build(key):
    f = dict(key[0])
    ws = dict(key[1])  # weight scales: wq wk wv wo w1 w2 g1 g2
    nc = bacc.Bacc(None, target_bir_lowering=False)

    dp = nc.declare_dram_parameter
    x_in = dp("x", [S, H], F32, isOutput=False)
    vol = dp("vol", [S], F32, isOutput=False)
    wq = dp("wq", [P, DC, 2, H], FP8, isOutput=False)
    wk = dp("wk", [P, DC, 2, H], FP8, isOutput=False)
    wv = dp("wv", [P, DC, 2, H], FP8, isOutput=False)
    wo = dp("wo", [P, DC, 2, H], FP8, isOutput=False)
    w1 = dp("w1", [P, DC, 2, HF], FP8, isOutput=False)
    w2 = dp("w2", [P, HF // 256, 2, H], FP8, isOutput=False)
    g1 = dp("g1", [P, DC, 2, HG], FP8, isOutput=False)
    g2 = dp("g2", [P, HG // 256, 2, H], FP8, isOutput=False)
    bqs = dp("bqs", [H], F32, isOutput=False)   # bq*QAS
    bks = dp("bks", [H], F32, isOutput=False)   # bk*QAS
    bvs = dp("bvs", [H], F32, isOutput=False)   # bv*VS
    bo = dp("bo", [H], F32, isOutput=False)
    b1 = dp("b1", [HF], F32, isOutput=False)
    b2 = dp("b2", [H], F32, isOutput=False)
    gb1s = dp("gb1s", [HG], F32, isOutput=False)  # gb1*RS
    gb2 = dp("gb2", [H], F32, isOutput=False)
    ln1w = dp("ln1w", [H], F32, isOutput=False)
    ln1b = dp("ln1b", [H], F32, isOutput=False)
    ln2w = dp("ln2w", [H], F32, isOutput=False)
    ln2b = dp("ln2b", [H], F32, isOutput=False)
    sc = {}
    for name in ("gamma1", "beta1", "vs1w", "vs1b", "gamma2", "beta2", "vs2w", "vs2b"):
        sc[name] = dp(name, [1], F32, isOutput=False)
    out = dp("out", [S, H], F32, isOutput=True)

    # derived eviction constants
    c_q = QAS / (XS * ws["wq"])
    c_k = QAS / (XS * ws["wk"])
    c_v = VS / (XS * ws["wv"])
    c_sc = 0.125 / (QAS * QAS)          # exp scale
    c_wo = 1.0 / (CS * ws["wo"])
    c_x1 = X1S
    c_g1 = RS / (YS * ws["g1"])
    c_g2 = 1.0 / (RS * ws["g2"])
    c_f1 = 1.0 / (YS * ws["w1"])
    c_f2 = 1.0 / ws["w2"]
    c_ctx = CS / VS

    def chunked(param):
        return param.rearrange("(c p) -> p c", p=P)

    with tile.TileContext(nc) as tc:
        from contextlib import ExitStack
        with ExitStack() as ctx:
            const = ctx.enter_context(tc.tile_pool(name="const", bufs=1))

            identf = const.tile([P, P], F32)
            make_identity(nc, identf)
            ones_row = const.tile([1, P], BF16)
            nc.vector.memset(ones_row, 1.0)
            ones_colb = const.tile([P, 1], BF16)
            nc.vector.memset(ones_colb, 1.0)
            ones_colq = const.tile([P, 1], FP8)
            nc.vector.memset(ones_colq, 1.0)
            eps128 = const.tile([P, 1], F32)
            nc.vector.memset(eps128, EPS)
            lnES = const.tile([P, 1], F32)
            nc.vector.memset(lnES, float(np.log(ES)))

            # persistent slabs (tag-shared across phases)
            trunk = ctx.enter_context(tc.tile_pool(name="trunk", bufs=1))
            xTf = trunk.tile([P, HC, S], F32, tag="f4a", name="xTf")
            QTs = trunk.tile([P, HC, S], FP8, tag="q8a", name="QTs")
            KTs = trunk.tile([P, HC, S], FP8, tag="q8b", name="KTs")
            Vpq = trunk.tile([P, DC, 2, NH, 66], FP8, tag="q8c", name="Vpq")
            xq = trunk.tile([P, DC, 2, S], FP8, tag="q8d", name="xq")
            ctxq = trunk.tile([P, DC, 2, S], FP8, tag="q8e", name="ctxq")
            accf = trunk.tile([P, HC, S], F32, tag="f4c", name="accf")

            # ---------------- P1: PE transpose of x ----------------
            with tc.tile_pool(name="p1", bufs=3) as p1, \
                 tc.tile_pool(name="p1ps", bufs=4, space="PSUM") as p1ps:
                identp = p1.tile([P, P], F32, name="identp", bufs=1)
                make_identity(nc, identp)
                for qc in range(HC):
                    xrow = p1.tile([P, H], F32, tag="xrow")
                    nc.sync.dma_start(out=xrow, in_=x_in[qc * P:(qc + 1) * P, :])
                    for hc in range(HC):
                        pst = p1ps.tile([P, P], F32, tag="ps_tr")
                        nc.tensor.transpose(pst, xrow[:, hc * P:(hc + 1) * P], identp)
                        nc.vector.tensor_copy(xTf[:, hc, qc * P:(qc + 1) * P], pst)
                    nc.scalar.activation(xq[:, :, :, qc * P:(qc + 1) * P],
                                         xTf[:, :, qc * P:(qc + 1) * P],
                                         AF.Identity, scale=XS)

            def load_chunked(param, n, name):
                t = const.tile([P, n], F32, name=name)
                nc.sync.dma_start(out=t, in_=chunked(param))
                return t

            bqs_sb = load_chunked(bqs, HC, "c_bqs")
            bks_sb = load_chunked(bks, HC, "c_bks")
            bo_sb = load_chunked(bo, HC, "c_bo") if f["bo"] else None
            b1_sb = load_chunked(b1, FC, "c_b1")
            b2_sb = load_chunked(b2, HC, "c_b2") if f["b2"] else None
            gb1s_sb = load_chunked(gb1s, GC, "c_gb1s")
            gb2_sb = load_chunked(gb2, HC, "c_gb2")
            if f["bv"]:
                bvs_bc = const.tile([P, H], F32)
                nc.gpsimd.dma_start(out=bvs_bc, in_=_bcast_ap(bvs, P, H))
            if f["ln1w"]:
                ln1w_sb = load_chunked(ln1w, HC, "c_ln1w")
            if f["ln1b"]:
                ln1b_sb = load_chunked(ln1b, HC, "c_ln1b")
            if f["ln2w"]:
                ln2w_bc = const.tile([P, H], F32)
                nc.gpsimd.dma_start(out=ln2w_bc, in_=_bcast_ap(ln2w, P, H))
            if f["ln2b"]:
                ln2b_bc = const.tile([P, H], F32)
                nc.gpsimd.dma_start(out=ln2b_bc, in_=_bcast_ap(ln2b, P, H))

            sct = {}
            for name in ("gamma1", "vs1w", "vs1b"):
                t = const.tile([1, 1], F32, name=f"sc_{name}")
                nc.sync.dma_start(out=t, in_=sc[name][None, :])
                sct[name] = t
            for name in ("gamma2", "beta2", "vs2w", "vs2b", "beta1"):
                t = const.tile([P, 1], F32, name=f"sc_{name}")
                nc.gpsimd.dma_start(out=t, in_=_bcast_ap(sc[name], P, 1))
                sct[name] = t

            vol_row = const.tile([1, S], F32)
            nc.sync.dma_start(out=vol_row, in_=vol[None, :])
            s1row = const.tile([1, S], F32)
            nc.scalar.activation(s1row, vol_row, AF.Sigmoid,
                                 bias=sct["vs1b"][0:1, :], scale=sct["vs1w"][0:1, :])
            nc.vector.tensor_scalar(s1row, s1row, 1.0, sct["gamma1"],
                                    op0=OP.add, op1=OP.mult)
            vol_np = const.tile([P, HC], F32)
            nc.sync.dma_start(out=vol_np, in_=chunked(vol))
            s2_np = const.tile([P, HC], F32)
            nc.scalar.activation(s2_np, vol_np, AF.Sigmoid,
                                 bias=sct["vs2b"], scale=sct["vs2w"])
            nc.vector.tensor_scalar(s2_np, s2_np, 1.0, sct["gamma2"],
                                    op0=OP.add, op1=OP.mult)

            # ---------------- P2: QKV projections (fp8 DR) ----------------
            nc.vector.memset(Vpq[:, :, :, :, 64:66], 1.0)
            with tc.tile_pool(name="p2w", bufs=3) as p2w, \
                 tc.tile_pool(name="p2wv", bufs=2) as p2wv, \
                 tc.tile_pool(name="p2ps", bufs=1, space="PSUM") as p2ps:
                for w_par, dst, bias_sb, cc in ((wq, QTs, bqs_sb, c_q),
                                                (wk, KTs, bks_sb, c_k)):
                    for mc in range(HC):
                        wt = p2w.tile([P, DC, 2, P], FP8, tag="wproj")
                        nc.sync.dma_start(out=wt,
                                          in_=w_par[:, :, :, mc * P:(mc + 1) * P])
                        ps = p2ps.tile([P, S], F32, tag="ps_qk", bufs=2)
                        for qt in range(QT):
                            for dcp in range(DC):
                                nc.tensor.matmul(ps[:, qt * 512:(qt + 1) * 512],
                                                 wt[:, dcp, :, :],
                                                 xq[:, dcp, :, qt * 512:(qt + 1) * 512],
                                                 start=(dcp == 0), stop=(dcp == DC - 1),
                                                 perf_mode=DRM)
                        nc.vector.tensor_scalar(dst[:, mc, :], ps, cc,
                                                bias_sb[:, mc:mc + 1],
                                                op0=OP.mult, op1=OP.add)
                for dt in range(2):
                    wt = p2wv.tile([P, DC, 2, 512], FP8, tag="wv")
                    nc.sync.dma_start(out=wt, in_=wv[:, :, :, dt * 512:(dt + 1) * 512])
                    for kc in range(HC):
                        ps = p2ps.tile([P, 512], F32, tag="ps_v", bufs=2)
                        for dcp in range(DC):
                            nc.tensor.matmul(ps, xq[:, dcp, :, kc * P:(kc + 1) * P],
                                             wt[:, dcp, :, :],
                                             start=(dcp == 0), stop=(dcp == DC - 1),
                                             perf_mode=DRM)
                        dst = Vpq[:, kc // 2, kc % 2, dt * 8:(dt + 1) * 8, 0:HD]
                        src = ps.rearrange("p (h d) -> p h d", d=HD)
                        if f["bv"]:
                            nc.vector.scalar_tensor_tensor(
                                dst, src, c_v,
                                bvs_bc[:, dt * 512:(dt + 1) * 512].rearrange(
                                    "p (h d) -> p h d", d=HD),
                                op0=OP.mult, op1=OP.add)
                        else:
                            nc.vector.tensor_scalar(dst, src, c_v, None, op0=OP.mult)

            # ---------------- P3: attention (fp8 scores + DR AV) ----------
            with tc.tile_pool(name="p3e", bufs=2) as p3e, \
                 tc.tile_pool(name="p3r", bufs=4) as p3r, \
                 tc.tile_pool(name="p3ps", bufs=1, space="PSUM") as p3ps:

                def scores(hp, epair):
                    for kc in range(HC):
                        tA = p3ps.tile([P, S], F32, tag="ps_s", bufs=2, name="tA")
                        tB = p3ps.tile([P, S], F32, tag="ps_s", bufs=2, name="tB")
                        for qt in range(QT):
                            sl = slice(qt * 512, (qt + 1) * 512)
                            nc.tensor.matmul(tA[:, sl],
                                             KTs[0:64, hp, kc * P:(kc + 1) * P],
                                             QTs[0:64, hp, sl],
                                             start=True, stop=True)
                            nc.tensor.matmul(tB[:, sl],
                                             KTs[64:128, hp, kc * P:(kc + 1) * P],
                                             QTs[64:128, hp, sl],
                                             start=True, stop=True)
                        nc.scalar.activation(epair[0][:, kc, :], tA, AF.Exp,
                                             scale=c_sc, bias=lnES)
                        nc.scalar.activation(epair[1][:, kc, :], tB, AF.Exp,
                                             scale=c_sc, bias=lnES)

                def av(hp, epair):
                    for j in range(2):
                        h = 2 * hp + j
                        p0 = 64 * j
                        e = epair[j]
                        for qt in range(QT):
                            sl = slice(qt * 512, (qt + 1) * 512)
                            pav = p3ps.tile([65, 512], F32, tag="ps_av", bufs=2,
                                            name="pav")
                            for kcp in range(DC):
                                nc.tensor.matmul(
                                    pav, Vpq[:, kcp, :, h, 0:65],
                                    e[:, 2 * kcp:2 * kcp + 2, sl],
                                    start=(kcp == 0), stop=(kcp == DC - 1),
                                    perf_mode=DRM)
                            rs = p3r.tile([1, 512], BF16, tag="rs")
                            nc.vector.tensor_copy(rs, pav[64:65, :])
                            pbc = p3ps.tile([64, 512], F32, tag="ps_bc", bufs=2,
                                            name="pbc")
                            nc.tensor.matmul(pbc, ones_row[0:1, 0:64], rs,
                                             start=True, stop=True)
                            rec = p3r.tile([64, 512], F32, tag="rec")
                            nc.vector.reciprocal_approx_fast(out=rec, in_=pbc)
                            nc.vector.scalar_tensor_tensor(
                                ctxq[p0:p0 + 64, hp // 2, hp % 2, sl],
                                rec, c_ctx, pav[0:64, :],
                                op0=OP.mult, op1=OP.mult)

                prev = None
                for hp in range(NH // 2):
                    epair = (p3e.tile([P, HC, S], FP8, tag="E0", name="e0"),
                             p3e.tile([P, HC, S], FP8, tag="E1", name="e1"))
                    scores(hp, epair)
                    if prev is not None:
                        av(hp - 1, prev)
                    prev = epair
                av(NH // 2 - 1, prev)

            # ---------------- P4+P5: Wo + residual + LN1 + gate1 ----------
            x1q = trunk.tile([P, DC, 2, S], FP8, tag="q8c", name="x1q")  # Vpq slot
            g1_bufs = 1 if f["ln1b"] else 2
            with tc.tile_pool(name="pw", bufs=3) as pw:
              with tc.tile_pool(name="pt4", bufs=1) as pt4, \
                   tc.tile_pool(name="pAps", bufs=1, space="PSUM") as pAps:
                for qt in range(QT):
                    sl = slice(qt * 512, (qt + 1) * 512)
                    for mc in range(HC):
                        wt = pw.tile([P, DC, 2, P], FP8, tag="wproj", bufs=3)
                        nc.sync.dma_start(out=wt, in_=wo[:, :, :, mc * P:(mc + 1) * P])
                        ps = pAps.tile([P, 512], F32, tag="ps_o", bufs=2)
                        for dcp in range(DC):
                            nc.tensor.matmul(ps, wt[:, dcp, :, :],
                                             ctxq[:, dcp, :, sl],
                                             start=(dcp == 0), stop=(dcp == DC - 1),
                                             perf_mode=DRM)
                        xs = xTf[:, mc, sl]
                        nc.vector.scalar_tensor_tensor(xs, ps, c_wo, xs,
                                                       op0=OP.mult, op1=OP.add)
                        if f["bo"]:
                            nc.vector.tensor_scalar(xs, xs, bo_sb[:, mc:mc + 1], None,
                                                    op0=OP.add)
                        nc.scalar.activation(x1q[:, mc // 2, mc % 2, sl], xs,
                                             AF.Identity, scale=c_x1)
                    # LN1 for this q-tile (fp32 trunk in place)
                    pstat = pAps.tile([33, 512], F32, tag="ps_stat")
                    for mc in range(HC):
                        nc.tensor.matmul(pstat[0:1, :], ones_colq,
                                         x1q[:, mc // 2, mc % 2, sl],
                                         start=(mc == 0), stop=(mc == HC - 1))
                    sq = pt4.tile([P, HC, 512], BF16, tag="sq")
                    nc.vector.tensor_tensor(sq, xTf[:, :, sl], xTf[:, :, sl], OP.mult)
                    for mc in range(HC):
                        nc.tensor.matmul(pstat[32:33, :], ones_colb, sq[:, mc, :],
                                         start=(mc == 0), stop=(mc == HC - 1))
                    mu = pt4.tile([1, 512], F32, tag="mu")
                    nc.vector.tensor_scalar(mu, pstat[0:1, :], 1.0 / (H * X1S), None,
                                            op0=OP.mult)
                    mu2 = pt4.tile([1, 512], F32, tag="mu2")
                    nc.vector.tensor_tensor(mu2, mu, mu, OP.mult)
                    var = pt4.tile([1, 512], F32, tag="var")
                    nc.vector.scalar_tensor_tensor(var, pstat[32:33, :], 1.0 / H, mu2,
                                                   op0=OP.mult, op1=OP.subtract)
                    nc.scalar.activation(var, var, AF.Sqrt, bias=eps128[0:1, :])
                    rstd = pt4.tile([1, 512], F32, tag="rstd")
                    nc.vector.reciprocal_approx_fast(out=rstd, in_=var)
                    arow = pt4.tile([1, 512], F32, tag="arow")
                    nc.vector.tensor_tensor(arow, rstd, s1row[0:1, sl], OP.mult)
                    arow_bf = pt4.tile([1, 512], BF16, tag="arow_bf")
                    nc.vector.tensor_copy(arow_bf, arow)
                    crow_bf = pt4.tile([1, 512], BF16, tag="crow_bf")
                    nc.vector.tensor_tensor(crow_bf, mu, arow, OP.mult)
                    psa = pAps.tile([P, 512], F32, tag="ps_a")
                    nc.tensor.matmul(psa, ones_row, arow_bf, start=True, stop=True)
                    psc = pAps.tile([P, 512], F32, tag="ps_c")
                    nc.tensor.matmul(psc, ones_row, crow_bf, start=True, stop=True)
                    if f["ln1b"]:
                        s1_bf = pt4.tile([1, 512], BF16, tag="s1_bf")
                        nc.vector.tensor_copy(s1_bf, s1row[0:1, sl])
                        pss1 = pAps.tile([P, 512], F32, tag="ps_s1")
                        nc.tensor.matmul(pss1, ones_row, s1_bf, start=True, stop=True)
                    for mc in range(HC):
                        y = xTf[:, mc, sl]
                        nc.vector.tensor_tensor(y, y, psa, OP.mult)
                        nc.vector.tensor_tensor(y, y, psc, OP.subtract)
                        if f["ln1w"]:
                            nc.vector.tensor_scalar(y, y, ln1w_sb[:, mc:mc + 1], None,
                                                    op0=OP.mult)
                        if f["ln1b"]:
                            bs = pt4.tile([P, 512], F32, tag="bs")
                            nc.vector.tensor_scalar(bs, pss1, ln1b_sb[:, mc:mc + 1],
                                                    None, op0=OP.mult)
                            nc.vector.tensor_tensor(y, y, bs, OP.add)
                        if f["beta1"]:
                            nc.vector.tensor_scalar(y, y, sct["beta1"], None, op0=OP.add)
                        # y1 (fp8) overwrites x1q in place
                        nc.scalar.activation(x1q[:, mc // 2, mc % 2, sl], y,
                                             AF.Identity, scale=YS)

                y1q = x1q
                rT = trunk.tile([P, HG // 256, 2, S], FP8, tag="q8d", name="rT")  # xq slot
                # gate first layer
                for qt in range(QT):
                    sl = slice(qt * 512, (qt + 1) * 512)
                    for mc in range(GC):
                        wt = pw.tile([P, DC, 2, P], FP8, tag="wproj", bufs=3)
                        nc.sync.dma_start(out=wt, in_=g1[:, :, :, mc * P:(mc + 1) * P])
                        ps = pAps.tile([P, 512], F32, tag="ps_g1", bufs=g1_bufs)
                        for dcp in range(DC):
                            nc.tensor.matmul(ps, wt[:, dcp, :, :],
                                             y1q[:, dcp, :, sl],
                                             start=(dcp == 0), stop=(dcp == DC - 1),
                                             perf_mode=DRM)
                        nc.scalar.activation(rT[:, mc // 2, mc % 2, sl], ps, AF.Relu,
                                             bias=gb1s_sb[:, mc:mc + 1], scale=c_g1)

              # ---------------- P6: gate2 + FFN + gated mix ----------------
              gT = trunk.tile([P, HC, S], BF16, tag="q8a", name="gT",
                              padded_shape=None)  # QTs slot (16KB > 8KB ok)
              with tc.tile_pool(name="pt7", bufs=2) as pt7, \
                   tc.tile_pool(name="pCps", bufs=1, space="PSUM") as pCps:
                    psk = [0]

                    def accps(shape):
                        t = pCps.tile(shape, F32, tag=f"ps_acc{psk[0] % 4}",
                                      name=f"psacc{psk[0] % 4}")
                        psk[0] += 1
                        return t

                    for qt in range(QT):
                        sl = slice(qt * 512, (qt + 1) * 512)
                        for mc in range(HC):
                            wt = pw.tile([P, HG // 256, 2, P], FP8, tag="wg2", bufs=3)
                            nc.sync.dma_start(out=wt,
                                              in_=g2[:, :, :, mc * P:(mc + 1) * P])
                            ps = accps([P, 512])
                            for rp in range(HG // 256):
                                nc.tensor.matmul(ps, wt[:, rp, :, :],
                                                 rT[:, rp, :, sl],
                                                 start=(rp == 0),
                                                 stop=(rp == HG // 256 - 1),
                                                 perf_mode=DRM)
                            nc.scalar.activation(gT[:, mc, sl], ps,
                                                 AF.Sigmoid, bias=gb2_sb[:, mc:mc + 1],
                                                 scale=c_g2)
                    for half in range(2):
                        if half == 0:
                            hH = trunk.tile([P, 8, 2, S], FP8, tag="q8b", name="hA")
                        else:
                            hH = trunk.tile([P, 8, 2, S], FP8, tag="q8e", name="hB")
                        for c in range(16):
                            cg = half * 16 + c
                            wt = pw.tile([P, DC, 2, P], FP8, tag="wproj", bufs=3)
                            nc.sync.dma_start(out=wt,
                                              in_=w1[:, :, :, cg * P:(cg + 1) * P])
                            psh = accps([P, S])
                            for qt in range(QT):
                                for dcp in range(DC):
                                    nc.tensor.matmul(
                                        psh[:, qt * 512:(qt + 1) * 512],
                                        wt[:, dcp, :, :],
                                        y1q[:, dcp, :, qt * 512:(qt + 1) * 512],
                                        start=(dcp == 0), stop=(dcp == DC - 1),
                                        perf_mode=DRM)
                            nc.scalar.activation(hH[:, c // 2, c % 2, :], psh, AF.Gelu,
                                                 bias=b1_sb[:, cg:cg + 1], scale=c_f1)
                        for oh in range(2):
                            accs = [accps([P, S]) for mc in range(4)]
                            for j in range(8):
                                jg = half * 8 + j
                                wt = pw.tile([P, 2, 512], FP8, tag="w2", bufs=6)
                                nc.sync.dma_start(
                                    out=wt,
                                    in_=w2[:, jg, :, oh * 512:(oh + 1) * 512])
                                for mc in range(4):
                                    for qt in range(QT):
                                        nc.tensor.matmul(
                                            accs[mc][:, qt * 512:(qt + 1) * 512],
                                            wt[:, :, mc * P:(mc + 1) * P],
                                            hH[:, j, :, qt * 512:(qt + 1) * 512],
                                            start=(j == 0), stop=(j == 7),
                                            perf_mode=DRM)
                            for mc in range(4):
                                mcg = oh * 4 + mc
                                for qt in range(QT):
                                    a = accf[:, mcg, qt * 512:(qt + 1) * 512]
                                    psl = accs[mc][:, qt * 512:(qt + 1) * 512]
                                    y = xTf[:, mcg, qt * 512:(qt + 1) * 512]
                                    if half == 0:
                                        nc.vector.scalar_tensor_tensor(
                                            a, psl, c_f2, y,
                                            op0=OP.mult, op1=OP.subtract)
                                    else:
                                        nc.vector.scalar_tensor_tensor(
                                            a, psl, c_f2, a, op0=OP.mult, op1=OP.add)
                                        if f["b2"]:
                                            nc.vector.tensor_scalar(
                                                a, a, b2_sb[:, mcg:mcg + 1], None,
                                                op0=OP.add)
                                        g = gT[:, mcg, qt * 512:(qt + 1) * 512]
                                        nc.vector.tensor_tensor(a, a, g, OP.mult)
                                        nc.vector.scalar_tensor_tensor(
                                            a, y, 2.0, a, op0=OP.mult, op1=OP.add)

                    # ---------------- P7: LN2 (N-layout) + output ------------
                    for qc in range(HC):
                        xt = pt7.tile([P, H], F32, tag="x2")
                        for hc in range(HC):
                            pst = accps([P, P])
                            nc.tensor.transpose(pst, accf[:, hc, qc * P:(qc + 1) * P],
                                                identf)
                            nc.scalar.activation(xt[:, hc * P:(hc + 1) * P], pst,
                                                 AF.Identity)
                        stats = pt7.tile([P, 2, nc.vector.BN_STATS_DIM], F32,
                                         tag="stats")
                        for sg in range(2):
                            nc.vector.bn_stats(stats[:, sg, :],
                                               xt[:, sg * 512:(sg + 1) * 512])
                        mv = pt7.tile([P, nc.vector.BN_AGGR_DIM], F32, tag="mv")
                        nc.vector.bn_aggr(mv, stats)
                        sd = pt7.tile([P, 1], F32, tag="sd")
                        nc.scalar.activation(sd, mv[:, 1:2], AF.Sqrt, bias=eps128)
                        rstd2 = pt7.tile([P, 1], F32, tag="rstd2")
                        nc.vector.reciprocal(rstd2, sd)
                        a2 = pt7.tile([P, 1], F32, tag="a2")
                        nc.vector.tensor_tensor(a2, rstd2, s2_np[:, qc:qc + 1], OP.mult)
                        ot = pt7.tile([P, H], F32, tag="ot")
                        nc.vector.tensor_scalar(ot, xt, mv[:, 0:1], a2,
                                                op0=OP.subtract, op1=OP.mult)
                        if f["ln2w"]:
                            nc.vector.tensor_tensor(ot, ot, ln2w_bc, OP.mult)
                        if f["ln2b"]:
                            bs2 = pt7.tile([P, H], F32, tag="bs2")
                            nc.vector.tensor_scalar(bs2, ln2b_bc, s2_np[:, qc:qc + 1],
                                                    None, op0=OP.mult)
                            nc.vector.tensor_tensor(ot, ot, bs2, OP.add)
                        if f["beta2"]:
                            nc.vector.tensor_scalar(ot, ot, sct["beta2"], None,
                                                    op0=OP.add)
                        nc.sync.dma_start(out=out[qc * P:(qc + 1) * P, :], in_=ot)

    nc.compile()
    return nc


def _pow2_scale(arr):
    am = float(np.max(np.abs(arr)))
    if am <= 0:
        return 1.0
    return float(2.0 ** np.floor(np.log2(200.0 / am)))


def _quant_dr(w, scale, kgroups):
    """[K, M] f32 -> [P, K//256, 2, M] fp8 with k = dc*256 + i*128 + p."""
    f8 = ml_dtypes.float8_e4m3fn
    K, M = w.shape
    q = np.clip(w * scale, -240.0, 240.0)
    q = q.reshape(K // 256, 2, P, M).transpose(2, 0, 1, 3)
    return np.ascontiguousarray(q.astype(f8))


def _prep(inputs):
    x = np.asarray(inputs["x"], np.float32)
    volat = np.asarray(inputs["volatility"], np.float32)

    raw = {}
    for name, key in (("wq", "Wq"), ("wk", "Wk"), ("wv", "Wv"), ("wo", "Wo"),
                      ("w1", "ffn_w1"), ("w2", "ffn_w2"),
                      ("g1", "gate_w1"), ("g2", "gate_w2")):
        raw[name] = np.asarray(inputs[key], np.float32)
    ws = {name: _pow2_scale(w) for name, w in raw.items()}

    shared = {name: _quant_dr(w, ws[name], None) for name, w in raw.items()}

    bq = np.asarray(inputs["bq"], np.float32)
    bk = np.asarray(inputs["bk"], np.float32)
    bv = np.asarray(inputs["bv"], np.float32)
    shared["bqs"] = np.ascontiguousarray(bq * QAS)
    shared["bks"] = np.ascontiguousarray(bk * QAS)
    shared["bvs"] = np.ascontiguousarray(bv * VS)
    gb1 = np.asarray(inputs["gate_b1"], np.float32)
    shared["gb1s"] = np.ascontiguousarray(gb1 * RS)
    for name, key in (("bo", "bo"), ("b1", "ffn_b1"), ("b2", "ffn_b2"),
                      ("gb2", "gate_b2"),
                      ("ln1w", "ln1_w"), ("ln1b", "ln1_b"),
                      ("ln2w", "ln2_w"), ("ln2b", "ln2_b")):
        shared[name] = np.ascontiguousarray(np.asarray(inputs[key], np.float32))
    for name, key in (("gamma1", "gamma1"), ("beta1", "beta1"),
                      ("vs1w", "vs1_w"), ("vs1b", "vs1_b"),
                      ("gamma2", "gamma2"), ("beta2", "beta2"),
                      ("vs2w", "vs2_w"), ("vs2b", "vs2_b")):
        shared[name] = np.asarray(inputs[key], np.float32).reshape(1)

    flags = (
        ("bv", bool(np.any(bv))),
        ("bo", bool(np.any(shared["bo"]))),
        ("b2", bool(np.any(shared["b2"]))),
        ("ln1w", bool(np.any(shared["ln1w"] != 1.0))),
        ("ln1b", bool(np.any(shared["ln1b"]))),
        ("beta1", bool(shared["beta1"][0] != 0.0)),
        ("ln2w", bool(np.any(shared["ln2w"] != 1.0))),
        ("ln2b", bool(np.any(shared["ln2b"]))),
        ("beta2", bool(shared["beta2"][0] != 0.0)),
    )
    wskey = tuple(sorted(ws.items()))

    in_maps = []
    for b in range(B):
        m = dict(shared)
        m["x"] = np.ascontiguousarray(x[b])
        m["vol"] = np.ascontiguousarray(volat[b])
        in_maps.append(m)
    return in_maps, (flags, wskey)


def _run(inputs, trace=False):
    in_maps, key = _prep(inputs)
    if key not in _BUILD_CACHE:
        _BUILD_CACHE[key] = _build(key)
    nc = _BUILD_CACHE[key]
    res = run_bass_kernel_spmd(nc, in_maps, core_ids=list(range(B)), trace=trace)
    outs = np.stack([res.results[b]["out"] for b in range(B)], axis=0)
    return outs.astype(np.float32), res


def kernel(**inputs) -> np.ndarray:
    out, _ = _run(inputs, trace=False)
    return out


# revision 3
# speedup vs baseline: 1.0616x; 1.0616x over previous
"""EnhancedTransformerBlock on 8 TRN2 NeuronCores — fp8 DoubleRow, fused pipeline.

Data-parallel over batch (1 element/core, no collectives). T-layout trunk
([feature, token], fp32 residuals); every large GEMM is fp8(e4m3) with
perf_mode=DoubleRow (weights host-quantized with per-tensor pow2 scales into
[P, K/256, 2, M] interleaved layout; activations quantized at eviction).

Structure:
  P1  : DMA x + PE transpose -> xTf (f32) + xq (fp8).
  P3  : fused QKV+attention pair pipeline. Both heads of a (kc, qt) score
        chunk share one [P,2,512] psum tile and ONE exp instruction, so the
        two K=64 matmuls dispatch together and pack onto PE row groups
        0-1/2-3. Q/K projection of pair hp+1 and the V projection are
        interleaved into the exp-bound window; AV (DoubleRow over token
        pairs, ones-column rowsum as a 65th output row) runs one pair
        behind, with the normalize lagged one group to keep PE dense.
  P4-6: qt-split pipeline: per q-half Wo+residual -> LN1 -> gate1 -> gate2
        -> FFN1 -> FFN2+gated mix, with PE transposes of finalized x2
        chunks interleaved behind each gated mix; LN2 (bn_stats on the
        transposed bf16 x2) + output DMA of the first q-half run under the
        second half's compute.
"""
import sys

sys.path.insert(0, '/opt/trn_rl_repo')

import numpy as np
import ml_dtypes

import concourse.bass as bass
import concourse.bacc as bacc
import concourse.tile as tile
from concourse import mybir
from concourse.bass_utils import run_bass_kernel_spmd
from concourse.masks import make_identity

F32 = mybir.dt.float32
BF16 = mybir.dt.bfloat16
FP8 = mybir.dt.float8e4
AF = mybir.ActivationFunctionType
OP = mybir.AluOpType
DRM = mybir.MatmulPerfMode.DoubleRow

P = 128
B, S, H = 8, 1024, 1024
NH, HD = 16, 64
HF, HG = 4 * H, H // 2
HC = H // P
DC = H // 256
FC = HF // P
GC = HG // P
QT = S // 512
EPS = 1e-5

XS = 16.0
QAS = 16.0
ES = 2.0
VS = 32.0
CS = 32.0
X1S = 16.0
YS = 16.0
RS = 32.0

_BUILD_CACHE = {}


def _bcast_ap(param, n_part, n_free):
    ap = param[None, :]
    return bass.AP(tensor=ap.tensor, offset=ap.offset, ap=[[0, n_part], [1, n_free]])


def _build(key):
    f = dict(key[0])
    ws = dict(key[1])
    nc = bacc.Bacc(None, target_bir_lowering=False)

    dp = nc.declare_dram_parameter
    x_in = dp("x", [S, H], F32, isOutput=False)
    vol = dp("vol", [S], F32, isOutput=False)
    wq = dp("wq", [P, DC, 2, H], FP8, isOutput=False)
    wk = dp("wk", [P, DC, 2, H], FP8, isOutput=False)
    wv = dp("wv", [P, DC, 2, H], FP8, isOutput=False)
    wo = dp("wo", [P, DC, 2, H], FP8, isOutput=False)
    w1 = dp("w1", [P, DC, 2, HF], FP8, isOutput=False)
    w2 = dp("w2", [P, HF // 256, 2, H], FP8, isOutput=False)
    g1 = dp("g1", [P, DC, 2, HG], FP8, isOutput=False)
    g2 = dp("g2", [P, HG // 256, 2, H], FP8, isOutput=False)
    bqs = dp("bqs", [H], F32, isOutput=False)
    bks = dp("bks", [H], F32, isOutput=False)
    bvs = dp("bvs", [H], F32, isOutput=False)
    bo = dp("bo", [H], F32, isOutput=False)
    b1 = dp("b1", [HF], F32, isOutput=False)
    b2 = dp("b2", [H], F32, isOutput=False)
    gb1s = dp("gb1s", [HG], F32, isOutput=False)
    gb2 = dp("gb2", [H], F32, isOutput=False)
    ln1w = dp("ln1w", [H], F32, isOutput=False)
    ln1b = dp("ln1b", [H], F32, isOutput=False)
    ln2w = dp("ln2w", [H], F32, isOutput=False)
    ln2b = dp("ln2b", [H], F32, isOutput=False)
    sc = {}
    for name in ("gamma1", "beta1", "vs1w", "vs1b", "gamma2", "beta2", "vs2w", "vs2b"):
        sc[name] = dp(name, [1], F32, isOutput=False)
    out = dp("out", [S, H], F32, isOutput=True)

    c_q = QAS / (XS * ws["wq"])
    c_k = QAS / (XS * ws["wk"])
    c_v = VS / (XS * ws["wv"])
    c_sc = 0.125 / (QAS * QAS)
    c_wo = 1.0 / (CS * ws["wo"])
    c_g1 = RS / (YS * ws["g1"])
    c_g2 = 1.0 / (RS * ws["g2"])
    c_f1 = 1.0 / (YS * ws["w1"])
    c_f2 = 1.0 / ws["w2"]
    c_ctx = CS / VS

    def chunked(param):
        return param.rearrange("(c p) -> p c", p=P)

    with tile.TileContext(nc) as tc:
        from contextlib import ExitStack
        with ExitStack() as ctx:
            const = ctx.enter_context(tc.tile_pool(name="const", bufs=1))

            identf = const.tile([P, P], F32)
            make_identity(nc, identf)
            ones_row = const.tile([1, P], BF16)
            nc.vector.memset(ones_row, 1.0)
            ones_colb = const.tile([P, 1], BF16)
            nc.vector.memset(ones_colb, 1.0)
            ones_colq = const.tile([P, 1], FP8)
            nc.vector.memset(ones_colq, 1.0)
            eps128 = const.tile([P, 1], F32)
            nc.vector.memset(eps128, EPS)
            lnES = const.tile([P, 1], F32)
            nc.vector.memset(lnES, float(np.log(ES)))

            trunk = ctx.enter_context(tc.tile_pool(name="trunk", bufs=1))
            xTf = trunk.tile([P, HC, S], F32, tag="f4a", name="xTf")
            QTs = trunk.tile([P, HC, S], FP8, tag="q8a", name="QTs")
            KTs = trunk.tile([P, HC, S], FP8, tag="q8b", name="KTs")
            Vpq = trunk.tile([P, DC, 2, NH, 66], FP8, tag="q8c", name="Vpq")
            xq = trunk.tile([P, DC, 2, S], FP8, tag="q8d", name="xq")
            ctxq = trunk.tile([P, DC, 2, S], FP8, tag="q8e", name="ctxq")
            accf = trunk.tile([P, HC, S], F32, tag="f4c", name="accf")

            # ---------------- P1: PE transpose of x ----------------
            with tc.tile_pool(name="p1", bufs=3) as p1, \
                 tc.tile_pool(name="p1ps", bufs=4, space="PSUM") as p1ps:
                identp = p1.tile([P, P], F32, name="identp", bufs=1)
                make_identity(nc, identp)
                for qc in range(HC):
                    xrow = p1.tile([P, H], F32, tag="xrow")
                    nc.sync.dma_start(out=xrow, in_=x_in[qc * P:(qc + 1) * P, :])
                    for hc in range(HC):
                        pst = p1ps.tile([P, P], F32, tag="ps_tr")
                        nc.tensor.transpose(pst, xrow[:, hc * P:(hc + 1) * P], identp)
                        nc.vector.tensor_copy(xTf[:, hc, qc * P:(qc + 1) * P], pst)
                    nc.scalar.activation(xq[:, :, :, qc * P:(qc + 1) * P],
                                         xTf[:, :, qc * P:(qc + 1) * P],
                                         AF.Identity, scale=XS)

            def load_chunked(param, n, name):
                t = const.tile([P, n], F32, name=name)
                nc.sync.dma_start(out=t, in_=chunked(param))
                return t

            bqs_sb = load_chunked(bqs, HC, "c_bqs")
            bks_sb = load_chunked(bks, HC, "c_bks")
            bo_sb = load_chunked(bo, HC, "c_bo") if f["bo"] else None
            b1_sb = load_chunked(b1, FC, "c_b1")
            b2_sb = load_chunked(b2, HC, "c_b2") if f["b2"] else None
            gb1s_sb = load_chunked(gb1s, GC, "c_gb1s")
            gb2_sb = load_chunked(gb2, HC, "c_gb2")
            if f["bv"]:
                bvs_bc = const.tile([P, H], F32)
                nc.gpsimd.dma_start(out=bvs_bc, in_=_bcast_ap(bvs, P, H))
            if f["ln1w"]:
                ln1w_sb = load_chunked(ln1w, HC, "c_ln1w")
            if f["ln1b"]:
                ln1b_sb = load_chunked(ln1b, HC, "c_ln1b")
            if f["ln2w"]:
                ln2w_bc = const.tile([P, H], F32)
                nc.gpsimd.dma_start(out=ln2w_bc, in_=_bcast_ap(ln2w, P, H))
            if f["ln2b"]:
                ln2b_bc = const.tile([P, H], F32)
                nc.gpsimd.dma_start(out=ln2b_bc, in_=_bcast_ap(ln2b, P, H))

            sct = {}
            for name in ("gamma1", "vs1w", "vs1b"):
                t = const.tile([1, 1], F32, name=f"sc_{name}")
                nc.sync.dma_start(out=t, in_=sc[name][None, :])
                sct[name] = t
            for name in ("gamma2", "beta2", "vs2w", "vs2b", "beta1"):
                t = const.tile([P, 1], F32, name=f"sc_{name}")
                nc.gpsimd.dma_start(out=t, in_=_bcast_ap(sc[name], P, 1))
                sct[name] = t

            vol_row = const.tile([1, S], F32)
            nc.sync.dma_start(out=vol_row, in_=vol[None, :])
            s1row = const.tile([1, S], F32)
            nc.scalar.activation(s1row, vol_row, AF.Sigmoid,
                                 bias=sct["vs1b"][0:1, :], scale=sct["vs1w"][0:1, :])
            nc.vector.tensor_scalar(s1row, s1row, 1.0, sct["gamma1"],
                                    op0=OP.add, op1=OP.mult)
            vol_np = const.tile([P, HC], F32)
            nc.sync.dma_start(out=vol_np, in_=chunked(vol))
            s2_np = const.tile([P, HC], F32)
            nc.scalar.activation(s2_np, vol_np, AF.Sigmoid,
                                 bias=sct["vs2b"], scale=sct["vs2w"])
            nc.vector.tensor_scalar(s2_np, s2_np, 1.0, sct["gamma2"],
                                    op0=OP.add, op1=OP.mult)

            s1_bc = None
            if f["ln1b"]:
                s1_bc = const.tile([P, S], F32)
                with tc.tile_pool(name="s1ps", bufs=1, space="PSUM") as s1ps:
                    s1_bf = const.tile([1, S], BF16)
                    nc.vector.tensor_copy(s1_bf, s1row)
                    for qt in range(QT):
                        pss1 = s1ps.tile([P, 512], F32, tag="pss1", bufs=2)
                        nc.tensor.matmul(pss1, ones_row,
                                         s1_bf[0:1, qt * 512:(qt + 1) * 512],
                                         start=True, stop=True)
                        nc.vector.tensor_copy(s1_bc[:, qt * 512:(qt + 1) * 512], pss1)

            # ---------------- P3: fused QKV + attention ----------------
            nc.vector.memset(Vpq[:, :, :, :, 64:66], 1.0)
            with tc.tile_pool(name="p2w", bufs=1) as p2w, \
                 tc.tile_pool(name="p3e", bufs=2) as p3e, \
                 tc.tile_pool(name="p3r", bufs=2) as p3r, \
                 tc.tile_pool(name="p3ps", bufs=1, space="PSUM") as p3ps:

                def projqk(hp):
                    for w_par, dst, bias_sb, cc in ((wq, QTs, bqs_sb, c_q),
                                                    (wk, KTs, bks_sb, c_k)):
                        wt = p2w.tile([P, DC, 2, P], FP8, tag="wqk", bufs=3)
                        nc.sync.dma_start(out=wt,
                                          in_=w_par[:, :, :, hp * P:(hp + 1) * P])
                        ps = p3ps.tile([P, 2, 512], F32, tag="psA", bufs=2,
                                       name="ps_qk")
                        for qt in range(QT):
                            for dcp in range(DC):
                                nc.tensor.matmul(
                                    ps[:, qt, :], wt[:, dcp, :, :],
                                    xq[:, dcp, :, qt * 512:(qt + 1) * 512],
                                    start=(dcp == 0), stop=(dcp == DC - 1),
                                    perf_mode=DRM)
                        nc.vector.tensor_scalar(dst[:, hp, :],
                                                ps.rearrange("p a b -> p (a b)"),
                                                cc, bias_sb[:, hp:hp + 1],
                                                op0=OP.mult, op1=OP.add)

                def projv(dt):
                    wt = p2w.tile([P, DC, 2, 512], FP8, tag="wv", bufs=2)
                    nc.sync.dma_start(out=wt, in_=wv[:, :, :, dt * 512:(dt + 1) * 512])
                    for kc in range(HC):
                        ps = p3ps.tile([P, 512], F32, tag="psV", bufs=1, name="ps_v")
                        for dcp in range(DC):
                            nc.tensor.matmul(ps, xq[:, dcp, :, kc * P:(kc + 1) * P],
                                             wt[:, dcp, :, :],
                                             start=(dcp == 0), stop=(dcp == DC - 1),
                                             perf_mode=DRM)
                        dst = Vpq[:, kc // 2, kc % 2, dt * 8:(dt + 1) * 8, 0:HD]
                        src = ps.rearrange("p (h d) -> p h d", d=HD)
                        if f["bv"]:
                            nc.vector.scalar_tensor_tensor(
                                dst, src, c_v,
                                bvs_bc[:, dt * 512:(dt + 1) * 512].rearrange(
                                    "p (h d) -> p h d", d=HD),
                                op0=OP.mult, op1=OP.add)
                        else:
                            nc.vector.tensor_scalar(dst, src, c_v, None, op0=OP.mult)

                def scores(hp, ep):
                    for kc in range(HC):
                        for qt in range(QT):
                            sl = slice(qt * 512, (qt + 1) * 512)
                            t2 = p3ps.tile([P, 2, 512], F32, tag="psA", bufs=2,
                                           name="t2")
                            nc.tensor.matmul(t2[:, 0, :],
                                             KTs[0:64, hp, kc * P:(kc + 1) * P],
                                             QTs[0:64, hp, sl],
                                             start=True, stop=True)
                            nc.tensor.matmul(t2[:, 1, :],
                                             KTs[64:128, hp, kc * P:(kc + 1) * P],
                                             QTs[64:128, hp, sl],
                                             start=True, stop=True)
                            nc.scalar.activation(ep[:, kc, :, sl], t2, AF.Exp,
                                                 scale=c_sc, bias=lnES)

                def _norm(hp, j, qt, pav, rs):
                    p0 = 64 * j
                    sl = slice(qt * 512, (qt + 1) * 512)
                    pbc = p3ps.tile([64, 512], F32, tag="psB", bufs=1, name="pbc")
                    nc.tensor.matmul(pbc, ones_row[0:1, 0:64], rs,
                                     start=True, stop=True)
                    rec = p3r.tile([64, 512], F32, tag="rec")
                    nc.vector.reciprocal_approx_fast(out=rec, in_=pbc)
                    nc.vector.scalar_tensor_tensor(
                        ctxq[p0:p0 + 64, hp // 2, hp % 2, sl],
                        rec, c_ctx, pav[0:64, :], op0=OP.mult, op1=OP.mult)

                def av(hp, ep):
                    pend = []
                    for j in range(2):
                        h = 2 * hp + j
                        for qt in range(QT):
                            sl = slice(qt * 512, (qt + 1) * 512)
                            pav = p3ps.tile([65, 512], F32, tag="psC", bufs=2,
                                            name="pav")
                            for kcp in range(DC):
                                nc.tensor.matmul(
                                    pav, Vpq[:, kcp, :, h, 0:65],
                                    ep[:, 2 * kcp:2 * kcp + 2, j, sl],
                                    start=(kcp == 0), stop=(kcp == DC - 1),
                                    perf_mode=DRM)
                            rs = p3r.tile([1, 512], BF16, tag="rs")
                            nc.vector.tensor_copy(rs, pav[64:65, :])
                            pend.append((j, qt, pav, rs))
                            if len(pend) >= 2:
                                _norm(hp, *pend.pop(0))
                    for g in pend:
                        _norm(hp, *g)

                projqk(0)
                projv(0)
                prev = None
                for hp in range(NH // 2):
                    ep = p3e.tile([P, HC, 2, S], FP8, tag="E", name="ep")
                    scores(hp, ep)
                    if hp < NH // 2 - 1:
                        projqk(hp + 1)
                    if hp == 1:
                        projv(1)
                    if prev is not None:
                        av(hp - 1, prev)
                    prev = ep
                av(NH // 2 - 1, prev)

            # ---------------- P4-P6: qt-split Wo/LN1/gate/FFN/LN2 ----------
            x1q = trunk.tile([P, DC, 2, S], FP8, tag="q8c", name="x1q")  # Vpq slot
            rT = trunk.tile([P, HG // 256, 2, S], FP8, tag="q8d", name="rT")  # xq slot
            with tc.tile_pool(name="pw", bufs=1) as pw, \
                 tc.tile_pool(name="pt4", bufs=1) as pt4, \
                 tc.tile_pool(name="big", bufs=1) as big, \
                 tc.tile_pool(name="pt7", bufs=2) as pt7, \
                 tc.tile_pool(name="pDps", bufs=1, space="PSUM") as pDps:

                gT = trunk.tile([P, HC, 512], BF16, tag="q8a", name="gT")  # QTs slot
                xt2 = big.tile([P, 4, H], BF16, name="xt2")
                hB = big.tile([P, 8, 2, S], FP8, name="hB")

                def wtile(shape):
                    return pDps.tile(shape, F32, tag="W", bufs=2, name="wt_ps")

                def wo_ln1(qt):
                    sl = slice(qt * 512, (qt + 1) * 512)
                    for mc in range(HC):
                        wt = pw.tile([P, DC, 2, P], FP8, tag="wproj", bufs=3)
                        nc.sync.dma_start(out=wt, in_=wo[:, :, :, mc * P:(mc + 1) * P])
                        ps = wtile([P, 512])
                        for dcp in range(DC):
                            nc.tensor.matmul(ps, wt[:, dcp, :, :],
                                             ctxq[:, dcp, :, sl],
                                             start=(dcp == 0), stop=(dcp == DC - 1),
                                             perf_mode=DRM)
                        xs = xTf[:, mc, sl]
                        nc.vector.scalar_tensor_tensor(xs, ps, c_wo, xs,
                                                       op0=OP.mult, op1=OP.add)
                        if f["bo"]:
                            nc.vector.tensor_scalar(xs, xs, bo_sb[:, mc:mc + 1], None,
                                                    op0=OP.add)
                        nc.scalar.activation(x1q[:, mc // 2, mc % 2, sl], xs,
                                             AF.Identity, scale=X1S)
                    pstat = pDps.tile([33, 512], F32, tag="ST", bufs=2, name="pstat")
                    for mc in range(HC):
                        nc.tensor.matmul(pstat[0:1, :], ones_colq,
                                         x1q[:, mc // 2, mc % 2, sl],
                                         start=(mc == 0), stop=(mc == HC - 1))
                    for mh in range(2):
                        sq = pt4.tile([P, 4, 512], BF16, tag="sq", bufs=2)
                        nc.vector.tensor_tensor(sq, xTf[:, 4 * mh:4 * mh + 4, sl],
                                                xTf[:, 4 * mh:4 * mh + 4, sl], OP.mult)
                        for mc in range(4):
                            nc.tensor.matmul(pstat[32:33, :], ones_colb, sq[:, mc, :],
                                             start=(mh == 0 and mc == 0),
                                             stop=(mh == 1 and mc == 3))
                    mu = pt4.tile([1, 512], F32, tag="mu")
                    nc.vector.tensor_scalar(mu, pstat[0:1, :], 1.0 / (H * X1S), None,
                                            op0=OP.mult)
                    mu2 = pt4.tile([1, 512], F32, tag="mu2")
                    nc.vector.tensor_tensor(mu2, mu, mu, OP.mult)
                    var = pt4.tile([1, 512], F32, tag="var")
                    nc.vector.scalar_tensor_tensor(var, pstat[32:33, :], 1.0 / H, mu2,
                                                   op0=OP.mult, op1=OP.subtract)
                    nc.scalar.activation(var, var, AF.Sqrt, bias=eps128[0:1, :])
                    rstd = pt4.tile([1, 512], F32, tag="rstd")
                    nc.vector.reciprocal_approx_fast(out=rstd, in_=var)
                    arow = pt4.tile([1, 512], F32, tag="arow")
                    nc.vector.tensor_tensor(arow, rstd, s1row[0:1, sl], OP.mult)
                    arow_bf = pt4.tile([1, 512], BF16, tag="arow_bf")
                    nc.vector.tensor_copy(arow_bf, arow)
                    crow_bf = pt4.tile([1, 512], BF16, tag="crow_bf")
                    nc.vector.tensor_tensor(crow_bf, mu, arow, OP.mult)
                    psa = wtile([P, 512])
                    nc.tensor.matmul(psa, ones_row, arow_bf, start=True, stop=True)
                    psc = wtile([P, 512])
                    nc.tensor.matmul(psc, ones_row, crow_bf, start=True, stop=True)
                    for mc in range(HC):
                        y = xTf[:, mc, sl]
                        nc.vector.tensor_tensor(y, y, psa, OP.mult)
                        nc.vector.tensor_tensor(y, y, psc, OP.subtract)
                        if f["ln1w"]:
                            nc.vector.tensor_scalar(y, y, ln1w_sb[:, mc:mc + 1], None,
                                                    op0=OP.mult)
                        if f["ln1b"]:
                            bs = pt4.tile([P, 512], F32, tag="bs")
                            nc.vector.tensor_scalar(bs, s1_bc[:, sl],
                                                    ln1b_sb[:, mc:mc + 1],
                                                    None, op0=OP.mult)
                            nc.vector.tensor_tensor(y, y, bs, OP.add)
                        if f["beta1"]:
                            nc.vector.tensor_scalar(y, y, sct["beta1"], None,
                                                    op0=OP.add)
                        nc.scalar.activation(x1q[:, mc // 2, mc % 2, sl], y,
                                             AF.Identity, scale=YS)

                def gates(qt):
                    sl = slice(qt * 512, (qt + 1) * 512)
                    for mc in range(GC):
                        wt = pw.tile([P, DC, 2, P], FP8, tag="wproj", bufs=3)
                        nc.sync.dma_start(out=wt, in_=g1[:, :, :, mc * P:(mc + 1) * P])
                        ps = wtile([P, 512])
                        for dcp in range(DC):
                            nc.tensor.matmul(ps, wt[:, dcp, :, :],
                                             x1q[:, dcp, :, sl],
                                             start=(dcp == 0), stop=(dcp == DC - 1),
                                             perf_mode=DRM)
                        nc.scalar.activation(rT[:, mc // 2, mc % 2, sl], ps, AF.Relu,
                                             bias=gb1s_sb[:, mc:mc + 1], scale=c_g1)
                    for mc in range(HC):
                        wt = pw.tile([P, HG // 256, 2, P], FP8, tag="wg2", bufs=3)
                        nc.sync.dma_start(out=wt, in_=g2[:, :, :, mc * P:(mc + 1) * P])
                        ps = wtile([P, 512])
                        for rp in range(HG // 256):
                            nc.tensor.matmul(ps, wt[:, rp, :, :], rT[:, rp, :, sl],
                                             start=(rp == 0),
                                             stop=(rp == HG // 256 - 1),
                                             perf_mode=DRM)
                        nc.scalar.activation(gT[:, mc, :], ps, AF.Sigmoid,
                                             bias=gb2_sb[:, mc:mc + 1], scale=c_g2)

                def ffn(qt, half, hH):
                    sl = slice(qt * 512, (qt + 1) * 512)
                    for c in range(16):
                        cg = half * 16 + c
                        wt = pw.tile([P, DC, 2, P], FP8, tag="wproj", bufs=3)
                        nc.sync.dma_start(out=wt, in_=w1[:, :, :, cg * P:(cg + 1) * P])
                        psh = wtile([P, 512])
                        for dcp in range(DC):
                            nc.tensor.matmul(psh, wt[:, dcp, :, :],
                                             x1q[:, dcp, :, sl],
                                             start=(dcp == 0), stop=(dcp == DC - 1),
                                             perf_mode=DRM)
                        nc.scalar.activation(hH[:, c // 2, c % 2, sl], psh, AF.Gelu,
                                             bias=b1_sb[:, cg:cg + 1], scale=c_f1)
                    for oh in range(2):
                        acc_t = []
                        for mc in range(4):
                            acc_t.append(pDps.tile([P, 512], F32, tag=f"acc{mc}",
                                                   bufs=1, name=f"acc{mc}"))
                        for j in range(8):
                            jg = half * 8 + j
                            wt = pw.tile([P, 2, 512], FP8, tag="w2", bufs=4)
                            nc.sync.dma_start(
                                out=wt, in_=w2[:, jg, :, oh * 512:(oh + 1) * 512])
                            for mc in range(4):
                                nc.tensor.matmul(acc_t[mc],
                                                 wt[:, :, mc * P:(mc + 1) * P],
                                                 hH[:, j, :, sl],
                                                 start=(j == 0), stop=(j == 7),
                                                 perf_mode=DRM)
                        for mc in range(4):
                            mcg = oh * 4 + mc
                            a = accf[:, mcg, sl]
                            psl = acc_t[mc]
                            y = xTf[:, mcg, sl]
                            if half == 0:
                                nc.vector.scalar_tensor_tensor(
                                    a, psl, c_f2, y, op0=OP.mult, op1=OP.subtract)
                            else:
                                nc.vector.scalar_tensor_tensor(
                                    a, psl, c_f2, a, op0=OP.mult, op1=OP.add)
                                if f["b2"]:
                                    nc.vector.tensor_scalar(
                                        a, a, b2_sb[:, mcg:mcg + 1], None, op0=OP.add)
                                g = gT[:, mcg, :]
                                nc.vector.tensor_tensor(a, a, g, OP.mult)
                                nc.vector.scalar_tensor_tensor(
                                    a, y, 2.0, a, op0=OP.mult, op1=OP.add)
                                for qi in range(4):
                                    qcc = qt * 4 + qi
                                    pst = wtile([P, P])
                                    nc.tensor.transpose(
                                        pst, accf[:, mcg, qcc * P:(qcc + 1) * P],
                                        identf)
                                    nc.scalar.activation(
                                        xt2[:, qi, mcg * P:(mcg + 1) * P], pst,
                                        AF.Identity)

                def ln2_out(qt):
                    for qi in range(4):
                        qc = qt * 4 + qi
                        xt = xt2[:, qi, :]
                        stats = pt7.tile([P, 2, nc.vector.BN_STATS_DIM], F32,
                                         tag="stats")
                        for sg in range(2):
                            nc.vector.bn_stats(stats[:, sg, :],
                                               xt[:, sg * 512:(sg + 1) * 512])
                        mv = pt7.tile([P, nc.vector.BN_AGGR_DIM], F32, tag="mv")
                        nc.vector.bn_aggr(mv, stats)
                        sd = pt7.tile([P, 1], F32, tag="sd")
                        nc.scalar.activation(sd, mv[:, 1:2], AF.Sqrt, bias=eps128)
                        rstd2 = pt7.tile([P, 1], F32, tag="rstd2")
                        nc.vector.reciprocal(rstd2, sd)
                        a2 = pt7.tile([P, 1], F32, tag="a2")
                        nc.vector.tensor_tensor(a2, rstd2, s2_np[:, qc:qc + 1],
                                                OP.mult)
                        ot = pt7.tile([P, H], F32, tag="ot", bufs=1)
                        nc.vector.tensor_scalar(ot, xt, mv[:, 0:1], a2,
                                                op0=OP.subtract, op1=OP.mult)
                        if f["ln2w"]:
                            nc.vector.tensor_tensor(ot, ot, ln2w_bc, OP.mult)
                        if f["ln2b"]:
                            bs2 = pt7.tile([P, H], F32, tag="bs2")
                            nc.vector.tensor_scalar(bs2, ln2b_bc, s2_np[:, qc:qc + 1],
                                                    None, op0=OP.mult)
                            nc.vector.tensor_tensor(ot, ot, bs2, OP.add)
                        if f["beta2"]:
                            nc.vector.tensor_scalar(ot, ot, sct["beta2"], None,
                                                    op0=OP.add)
                        nc.sync.dma_start(out=out[qc * P:(qc + 1) * P, :], in_=ot)

                for qt in range(QT):
                    wo_ln1(qt)
                    gates(qt)
                    for half in range(2):
                        if half == 0:
                            hH = trunk.tile([P, 8, 2, S], FP8, tag="q8b", name="hA")
                        else:
                            hH = hB
                        ffn(qt, half, hH)
                    ln2_out(qt)

    nc.compile()
    return nc


def _pow2_scale(arr):
    am = float(np.max(np.abs(arr)))
    if am <= 0:
        return 1.0
    return float(2.0 ** np.floor(np.log2(200.0 / am)))


def _quant_dr(w, scale):
    f8 = ml_dtypes.float8_e4m3fn
    K, M = w.shape
    q = np.clip(w * scale, -240.0, 240.0)
    q = q.reshape(K // 256, 2, P, M).transpose(2, 0, 1, 3)
    return np.ascontiguousarray(q.astype(f8))


def _prep(inputs):
    x = np.asarray(inputs["x"], np.float32)
    volat = np.asarray(inputs["volatility"], np.float32)

    raw = {}
    for name, key in (("wq", "Wq"), ("wk", "Wk"), ("wv", "Wv"), ("wo", "Wo"),
                      ("w1", "ffn_w1"), ("w2", "ffn_w2"),
                      ("g1", "gate_w1"), ("g2", "gate_w2")):
        raw[name] = np.asarray(inputs[key], np.float32)
    ws = {name: _pow2_scale(w) for name, w in raw.items()}
    shared = {name: _quant_dr(w, ws[name]) for name, w in raw.items()}

    bq = np.asarray(inputs["bq"], np.float32)
    bk = np.asarray(inputs["bk"], np.float32)
    bv = np.asarray(inputs["bv"], np.float32)
    shared["bqs"] = np.ascontiguousarray(bq * QAS)
    shared["bks"] = np.ascontiguousarray(bk * QAS)
    shared["bvs"] = np.ascontiguousarray(bv * VS)
    gb1 = np.asarray(inputs["gate_b1"], np.float32)
    shared["gb1s"] = np.ascontiguousarray(gb1 * RS)
    for name, key in (("bo", "bo"), ("b1", "ffn_b1"), ("b2", "ffn_b2"),
                      ("gb2", "gate_b2"),
                      ("ln1w", "ln1_w"), ("ln1b", "ln1_b"),
                      ("ln2w", "ln2_w"), ("ln2b", "ln2_b")):
        shared[name] = np.ascontiguousarray(np.asarray(inputs[key], np.float32))
    for name, key in (("gamma1", "gamma1"), ("beta1", "beta1"),
                      ("vs1w", "vs1_w"), ("vs1b", "vs1_b"),
                      ("gamma2", "gamma2"), ("beta2", "beta2"),
                      ("vs2w", "vs2_w"), ("vs2b", "vs2_b")):
        shared[name] = np.asarray(inputs[key], np.float32).reshape(1)

    flags = (
        ("bv", bool(np.any(bv))),
        ("bo", bool(np.any(shared["bo"]))),
        ("b2", bool(np.any(shared["b2"]))),
        ("ln1w", bool(np.any(shared["ln1w"] != 1.0))),
        ("ln1b", bool(np.any(shared["ln1b"]))),
        ("beta1", bool(shared["beta1"][0] != 0.0)),
        ("ln2w", bool(np.any(shared["ln2w"] != 1.0))),
        ("ln2b", bool(np.any(shared["ln2b"]))),
        ("beta2", bool(shared["beta2"][0] != 0.0)),
    )
    wskey = tuple(sorted(ws.items()))

    in_maps = []
    for b in range(B):
        m = dict(shared)
        m["x"] = np.ascontiguousarray(x[b])
        m["vol"] = np.ascontiguousarray(volat[b])
        in_maps.append(m)
    return in_maps, (flags, wskey)


def _run(inputs, trace=False):
    in_maps, key = _prep(inputs)
    if key not in _BUILD_CACHE:
        _BUILD_CACHE[key] = _build(key)
    nc = _BUILD_CACHE[key]
    res = run_bass_kernel_spmd(nc, in_maps, core_ids=list(range(B)), trace=trace)
    outs = np.stack([res.results[b]["out"] for b in range(B)], axis=0)
    return outs.astype(np.float32), res


def kernel(**inputs) -> np.ndarray:
    out, _ = _run(inputs, trace=False)
    return out


# revision 4
# speedup vs baseline: 1.0822x; 1.0194x over previous
"""EnhancedTransformerBlock on 8 TRN2 NeuronCores — fp8 DoubleRow, fused pipeline.

Data-parallel over batch (1 element/core, no collectives). T-layout trunk
([feature, token], fp32 residuals); every large GEMM is fp8(e4m3) with
perf_mode=DoubleRow (weights host-quantized with per-tensor pow2 scales into
[P, K/256, 2, M] interleaved layout; activations quantized at eviction).

Structure:
  P1  : DMA x + PE transpose -> xTf (f32) + xq (fp8).
  P3  : fused QKV+attention pair pipeline. Both heads of a (kc, qt) score
        chunk share one [P,2,512] psum tile and ONE exp instruction, so the
        two K=64 matmuls dispatch together and pack onto PE row groups
        0-1/2-3. Q/K projection of pair hp+1 and the V projection are
        interleaved into the exp-bound window; AV (DoubleRow over token
        pairs, ones-column rowsum as a 65th output row) runs one pair
        behind, with the normalize lagged one group to keep PE dense.
  P4-6: qt-split pipeline: per q-half Wo+residual -> LN1 -> gate1 -> gate2
        -> FFN1 -> FFN2+gated mix, with PE transposes of finalized x2
        chunks interleaved behind each gated mix; LN2 (bn_stats on the
        transposed bf16 x2) + output DMA of the first q-half run under the
        second half's compute.
"""
import sys

sys.path.insert(0, '/opt/trn_rl_repo')

import numpy as np
import ml_dtypes

import concourse.bass as bass
import concourse.bacc as bacc
import concourse.tile as tile
from concourse import mybir
from concourse.bass_utils import run_bass_kernel_spmd
from concourse.masks import make_identity

F32 = mybir.dt.float32
BF16 = mybir.dt.bfloat16
FP8 = mybir.dt.float8e4
AF = mybir.ActivationFunctionType
OP = mybir.AluOpType
DRM = mybir.MatmulPerfMode.DoubleRow

P = 128
B, S, H = 8, 1024, 1024
NH, HD = 16, 64
HF, HG = 4 * H, H // 2
HC = H // P
DC = H // 256
FC = HF // P
GC = HG // P
QT = S // 512
EPS = 1e-5

XS = 16.0
QAS = 16.0
ES = 2.0
VS = 32.0
CS = 32.0
X1S = 16.0
YS = 16.0
RS = 32.0

_BUILD_CACHE = {}


def _bcast_ap(param, n_part, n_free):
    ap = param[None, :]
    return bass.AP(tensor=ap.tensor, offset=ap.offset, ap=[[0, n_part], [1, n_free]])


def _build(key):
    f = dict(key[0])
    ws = dict(key[1])
    nc = bacc.Bacc(None, target_bir_lowering=False)

    dp = nc.declare_dram_parameter
    x_in = dp("x", [S, H], F32, isOutput=False)
    vol = dp("vol", [S], F32, isOutput=False)
    wq = dp("wq", [P, DC, 2, H], FP8, isOutput=False)
    wk = dp("wk", [P, DC, 2, H], FP8, isOutput=False)
    wv = dp("wv", [P, DC, 2, H], FP8, isOutput=False)
    wo = dp("wo", [P, DC, 2, H], FP8, isOutput=False)
    w1 = dp("w1", [P, DC, 2, HF], FP8, isOutput=False)
    w2 = dp("w2", [P, HF // 256, 2, H], FP8, isOutput=False)
    g1 = dp("g1", [P, DC, 2, HG], FP8, isOutput=False)
    g2 = dp("g2", [P, HG // 256, 2, H], FP8, isOutput=False)
    bqs = dp("bqs", [H], F32, isOutput=False)
    bks = dp("bks", [H], F32, isOutput=False)
    bvs = dp("bvs", [H], F32, isOutput=False)
    bo = dp("bo", [H], F32, isOutput=False)
    b1 = dp("b1", [HF], F32, isOutput=False)
    b2 = dp("b2", [H], F32, isOutput=False)
    gb1s = dp("gb1s", [HG], F32, isOutput=False)
    gb2 = dp("gb2", [H], F32, isOutput=False)
    ln1w = dp("ln1w", [H], F32, isOutput=False)
    ln1b = dp("ln1b", [H], F32, isOutput=False)
    ln2w = dp("ln2w", [H], F32, isOutput=False)
    ln2b = dp("ln2b", [H], F32, isOutput=False)
    sc = {}
    for name in ("gamma1", "beta1", "vs1w", "vs1b", "gamma2", "beta2", "vs2w", "vs2b"):
        sc[name] = dp(name, [1], F32, isOutput=False)
    out = dp("out", [S, H], F32, isOutput=True)

    c_q = QAS / (XS * ws["wq"])
    c_k = QAS / (XS * ws["wk"])
    c_v = VS / (XS * ws["wv"])
    c_sc = 0.125 / (QAS * QAS)
    c_wo = 1.0 / (CS * ws["wo"])
    c_g1 = RS / (YS * ws["g1"])
    c_g2 = 1.0 / (RS * ws["g2"])
    c_f1 = 1.0 / (YS * ws["w1"])
    c_f2 = 1.0 / ws["w2"]
    c_ctx = CS / VS

    def chunked(param):
        return param.rearrange("(c p) -> p c", p=P)

    with tile.TileContext(nc) as tc:
        from contextlib import ExitStack
        with ExitStack() as ctx:
            const = ctx.enter_context(tc.tile_pool(name="const", bufs=1))

            identf = const.tile([P, P], F32)
            make_identity(nc, identf)
            ones_row = const.tile([1, P], BF16)
            nc.vector.memset(ones_row, 1.0)
            ones_colb = const.tile([P, 1], BF16)
            nc.vector.memset(ones_colb, 1.0)
            ones_colq = const.tile([P, 1], FP8)
            nc.vector.memset(ones_colq, 1.0)
            eps128 = const.tile([P, 1], F32)
            nc.vector.memset(eps128, EPS)
            lnES = const.tile([P, 1], F32)
            nc.vector.memset(lnES, float(np.log(ES)))

            trunk = ctx.enter_context(tc.tile_pool(name="trunk", bufs=1))
            xTf = trunk.tile([P, HC, S], F32, tag="f4a", name="xTf")
            QTs = trunk.tile([P, HC, S], FP8, tag="q8a", name="QTs")
            KTs = trunk.tile([P, HC, S], FP8, tag="q8b", name="KTs")
            Vpq = trunk.tile([P, DC, 2, NH, 66], FP8, tag="q8c", name="Vpq")
            xq = trunk.tile([P, DC, 2, S], FP8, tag="q8d", name="xq")
            ctxq = trunk.tile([P, DC, 2, S], FP8, tag="q8e", name="ctxq")
            accf = trunk.tile([P, HC, S], F32, tag="f4c", name="accf")

            # ---------------- P1: PE transpose of x ----------------
            with tc.tile_pool(name="p1", bufs=3) as p1, \
                 tc.tile_pool(name="p1ps", bufs=4, space="PSUM") as p1ps:
                identp = p1.tile([P, P], F32, name="identp", bufs=1)
                make_identity(nc, identp)
                for qc in range(HC):
                    xrow = p1.tile([P, H], F32, tag="xrow")
                    nc.sync.dma_start(out=xrow, in_=x_in[qc * P:(qc + 1) * P, :])
                    for hc in range(HC):
                        pst = p1ps.tile([P, P], F32, tag="ps_tr")
                        nc.tensor.transpose(pst, xrow[:, hc * P:(hc + 1) * P], identp)
                        nc.vector.tensor_copy(xTf[:, hc, qc * P:(qc + 1) * P], pst)
                    nc.scalar.activation(xq[:, :, :, qc * P:(qc + 1) * P],
                                         xTf[:, :, qc * P:(qc + 1) * P],
                                         AF.Identity, scale=XS)

            def load_chunked(param, n, name):
                t = const.tile([P, n], F32, name=name)
                nc.sync.dma_start(out=t, in_=chunked(param))
                return t

            bqs_sb = load_chunked(bqs, HC, "c_bqs")
            bks_sb = load_chunked(bks, HC, "c_bks")
            bo_sb = load_chunked(bo, HC, "c_bo") if f["bo"] else None
            b1_sb = load_chunked(b1, FC, "c_b1")
            b2_sb = load_chunked(b2, HC, "c_b2") if f["b2"] else None
            gb1s_sb = load_chunked(gb1s, GC, "c_gb1s")
            gb2_sb = load_chunked(gb2, HC, "c_gb2")
            if f["bv"]:
                bvs_bc = const.tile([P, H], F32)
                nc.gpsimd.dma_start(out=bvs_bc, in_=_bcast_ap(bvs, P, H))
            if f["ln1w"]:
                ln1w_sb = load_chunked(ln1w, HC, "c_ln1w")
            if f["ln1b"]:
                ln1b_sb = load_chunked(ln1b, HC, "c_ln1b")
            if f["ln2w"]:
                ln2w_bc = const.tile([P, H], F32)
                nc.gpsimd.dma_start(out=ln2w_bc, in_=_bcast_ap(ln2w, P, H))
            if f["ln2b"]:
                ln2b_bc = const.tile([P, H], F32)
                nc.gpsimd.dma_start(out=ln2b_bc, in_=_bcast_ap(ln2b, P, H))

            sct = {}
            for name in ("gamma1", "vs1w", "vs1b"):
                t = const.tile([1, 1], F32, name=f"sc_{name}")
                nc.sync.dma_start(out=t, in_=sc[name][None, :])
                sct[name] = t
            for name in ("gamma2", "beta2", "vs2w", "vs2b", "beta1"):
                t = const.tile([P, 1], F32, name=f"sc_{name}")
                nc.gpsimd.dma_start(out=t, in_=_bcast_ap(sc[name], P, 1))
                sct[name] = t

            vol_row = const.tile([1, S], F32)
            nc.sync.dma_start(out=vol_row, in_=vol[None, :])
            s1row = const.tile([1, S], F32)
            nc.scalar.activation(s1row, vol_row, AF.Sigmoid,
                                 bias=sct["vs1b"][0:1, :], scale=sct["vs1w"][0:1, :])
            nc.vector.tensor_scalar(s1row, s1row, 1.0, sct["gamma1"],
                                    op0=OP.add, op1=OP.mult)
            vol_np = const.tile([P, HC], F32)
            nc.sync.dma_start(out=vol_np, in_=chunked(vol))
            s2_np = const.tile([P, HC], F32)
            nc.scalar.activation(s2_np, vol_np, AF.Sigmoid,
                                 bias=sct["vs2b"], scale=sct["vs2w"])
            nc.vector.tensor_scalar(s2_np, s2_np, 1.0, sct["gamma2"],
                                    op0=OP.add, op1=OP.mult)

            s1_bc = None
            if f["ln1b"]:
                s1_bc = const.tile([P, S], F32)
                with tc.tile_pool(name="s1ps", bufs=1, space="PSUM") as s1ps:
                    s1_bf = const.tile([1, S], BF16)
                    nc.vector.tensor_copy(s1_bf, s1row)
                    for qt in range(QT):
                        pss1 = s1ps.tile([P, 512], F32, tag="pss1", bufs=2)
                        nc.tensor.matmul(pss1, ones_row,
                                         s1_bf[0:1, qt * 512:(qt + 1) * 512],
                                         start=True, stop=True)
                        nc.vector.tensor_copy(s1_bc[:, qt * 512:(qt + 1) * 512], pss1)

            # ---------------- P3: fused QKV + attention ----------------
            nc.vector.memset(Vpq[:, :, :, :, 64:66], 1.0)
            with tc.tile_pool(name="p2w", bufs=1) as p2w, \
                 tc.tile_pool(name="p3e", bufs=2) as p3e, \
                 tc.tile_pool(name="p3r", bufs=2) as p3r, \
                 tc.tile_pool(name="p3ps", bufs=1, space="PSUM") as p3ps:

                def projqk(hp):
                    for w_par, dst, bias_sb, cc in ((wq, QTs, bqs_sb, c_q),
                                                    (wk, KTs, bks_sb, c_k)):
                        wt = p2w.tile([P, DC, 2, P], FP8, tag="wqk", bufs=3)
                        nc.sync.dma_start(out=wt,
                                          in_=w_par[:, :, :, hp * P:(hp + 1) * P])
                        ps = p3ps.tile([P, 2, 512], F32, tag="psA", bufs=2,
                                       name="ps_qk")
                        for qt in range(QT):
                            for dcp in range(DC):
                                nc.tensor.matmul(
                                    ps[:, qt, :], wt[:, dcp, :, :],
                                    xq[:, dcp, :, qt * 512:(qt + 1) * 512],
                                    start=(dcp == 0), stop=(dcp == DC - 1),
                                    perf_mode=DRM)
                        nc.vector.tensor_scalar(dst[:, hp, :],
                                                ps.rearrange("p a b -> p (a b)"),
                                                cc, bias_sb[:, hp:hp + 1],
                                                op0=OP.mult, op1=OP.add)

                def projv(dt):
                    wt = p2w.tile([P, DC, 2, 512], FP8, tag="wv", bufs=2)
                    nc.sync.dma_start(out=wt, in_=wv[:, :, :, dt * 512:(dt + 1) * 512])
                    for kc in range(HC):
                        ps = p3ps.tile([P, 512], F32, tag="psV", bufs=1, name="ps_v")
                        for dcp in range(DC):
                            nc.tensor.matmul(ps, xq[:, dcp, :, kc * P:(kc + 1) * P],
                                             wt[:, dcp, :, :],
                                             start=(dcp == 0), stop=(dcp == DC - 1),
                                             perf_mode=DRM)
                        dst = Vpq[:, kc // 2, kc % 2, dt * 8:(dt + 1) * 8, 0:HD]
                        src = ps.rearrange("p (h d) -> p h d", d=HD)
                        if f["bv"]:
                            nc.vector.scalar_tensor_tensor(
                                dst, src, c_v,
                                bvs_bc[:, dt * 512:(dt + 1) * 512].rearrange(
                                    "p (h d) -> p h d", d=HD),
                                op0=OP.mult, op1=OP.add)
                        else:
                            nc.vector.tensor_scalar(dst, src, c_v, None, op0=OP.mult)

                def scores(hp, ep):
                    for kc in range(HC):
                        for qt in range(QT):
                            sl = slice(qt * 512, (qt + 1) * 512)
                            t2 = p3ps.tile([P, 2, 512], F32, tag="psA", bufs=2,
                                           name="t2")
                            nc.tensor.matmul(t2[:, 0, :],
                                             KTs[0:64, hp, kc * P:(kc + 1) * P],
                                             QTs[0:64, hp, sl],
                                             start=True, stop=True)
                            nc.tensor.matmul(t2[:, 1, :],
                                             KTs[64:128, hp, kc * P:(kc + 1) * P],
                                             QTs[64:128, hp, sl],
                                             start=True, stop=True)
                            nc.scalar.activation(ep[:, kc, :, sl], t2, AF.Exp,
                                                 scale=c_sc, bias=lnES)

                def _norm(hp, j, qt, pav, rs):
                    p0 = 64 * j
                    sl = slice(qt * 512, (qt + 1) * 512)
                    pbc = p3ps.tile([64, 512], F32, tag="psB", bufs=1, name="pbc")
                    nc.tensor.matmul(pbc, ones_row[0:1, 0:64], rs,
                                     start=True, stop=True)
                    rec = p3r.tile([64, 512], F32, tag="rec")
                    nc.vector.reciprocal_approx_fast(out=rec, in_=pbc)
                    nc.vector.scalar_tensor_tensor(
                        ctxq[p0:p0 + 64, hp // 2, hp % 2, sl],
                        rec, c_ctx, pav[0:64, :], op0=OP.mult, op1=OP.mult)

                def av(hp, ep):
                    pend = []
                    for j in range(2):
                        h = 2 * hp + j
                        for qt in range(QT):
                            sl = slice(qt * 512, (qt + 1) * 512)
                            pav = p3ps.tile([65, 512], F32, tag="psC", bufs=2,
                                            name="pav")
                            for kcp in range(DC):
                                nc.tensor.matmul(
                                    pav, Vpq[:, kcp, :, h, 0:65],
                                    ep[:, 2 * kcp:2 * kcp + 2, j, sl],
                                    start=(kcp == 0), stop=(kcp == DC - 1),
                                    perf_mode=DRM)
                            rs = p3r.tile([1, 512], BF16, tag="rs")
                            nc.vector.tensor_copy(rs, pav[64:65, :])
                            pend.append((j, qt, pav, rs))
                            if len(pend) >= 2:
                                _norm(hp, *pend.pop(0))
                    for g in pend:
                        _norm(hp, *g)

                projqk(0)
                projv(0)
                prev = None
                for hp in range(NH // 2):
                    ep = p3e.tile([P, HC, 2, S], FP8, tag="E", name="ep")
                    scores(hp, ep)
                    if hp < NH // 2 - 1:
                        projqk(hp + 1)
                    if hp == 1:
                        projv(1)
                    if prev is not None:
                        av(hp - 1, prev)
                    prev = ep
                av(NH // 2 - 1, prev)

            # ---------------- P4-P6: qt-split Wo/LN1/gate/FFN/LN2 ----------
            x1q = trunk.tile([P, DC, 2, S], FP8, tag="q8c", name="x1q")  # Vpq slot
            rT = trunk.tile([P, HG // 256, 2, S], FP8, tag="q8d", name="rT")  # xq slot
            with tc.tile_pool(name="pw", bufs=1) as pw, \
                 tc.tile_pool(name="pt4", bufs=1) as pt4, \
                 tc.tile_pool(name="big", bufs=1) as big, \
                 tc.tile_pool(name="pt7", bufs=2) as pt7, \
                 tc.tile_pool(name="pDps", bufs=1, space="PSUM") as pDps:

                gT = trunk.tile([P, HC, 512], BF16, tag="q8a", name="gT")  # QTs slot
                xt2 = big.tile([P, 4, H], BF16, name="xt2")
                hB = big.tile([P, 8, 2, S], FP8, name="hB")

                def wtile(shape):
                    return pDps.tile(shape, F32, tag="W", bufs=2, name="wt_ps")

                def wo_half(qt):
                    sl = slice(qt * 512, (qt + 1) * 512)
                    for mc in range(HC):
                        wt = pw.tile([P, DC, 2, P], FP8, tag="wproj", bufs=3)
                        nc.sync.dma_start(out=wt, in_=wo[:, :, :, mc * P:(mc + 1) * P])
                        ps = wtile([P, 512])
                        for dcp in range(DC):
                            nc.tensor.matmul(ps, wt[:, dcp, :, :],
                                             ctxq[:, dcp, :, sl],
                                             start=(dcp == 0), stop=(dcp == DC - 1),
                                             perf_mode=DRM)
                        xs = xTf[:, mc, sl]
                        nc.vector.scalar_tensor_tensor(xs, ps, c_wo, xs,
                                                       op0=OP.mult, op1=OP.add)
                        if f["bo"]:
                            nc.vector.tensor_scalar(xs, xs, bo_sb[:, mc:mc + 1], None,
                                                    op0=OP.add)
                        nc.scalar.activation(x1q[:, mc // 2, mc % 2, sl], xs,
                                             AF.Identity, scale=X1S)

                def ln1(qt):
                    sl = slice(qt * 512, (qt + 1) * 512)
                    pstat = pDps.tile([33, 512], F32, tag="ST", bufs=2, name="pstat")
                    for mc in range(HC):
                        nc.tensor.matmul(pstat[0:1, :], ones_colq,
                                         x1q[:, mc // 2, mc % 2, sl],
                                         start=(mc == 0), stop=(mc == HC - 1))
                    for mh in range(2):
                        sq = pt4.tile([P, 4, 512], BF16, tag="sq", bufs=2)
                        nc.vector.tensor_tensor(sq, xTf[:, 4 * mh:4 * mh + 4, sl],
                                                xTf[:, 4 * mh:4 * mh + 4, sl], OP.mult)
                        for mc in range(4):
                            nc.tensor.matmul(pstat[32:33, :], ones_colb, sq[:, mc, :],
                                             start=(mh == 0 and mc == 0),
                                             stop=(mh == 1 and mc == 3))
                    mu = pt4.tile([1, 512], F32, tag="mu")
                    nc.vector.tensor_scalar(mu, pstat[0:1, :], 1.0 / (H * X1S), None,
                                            op0=OP.mult)
                    mu2 = pt4.tile([1, 512], F32, tag="mu2")
                    nc.vector.tensor_tensor(mu2, mu, mu, OP.mult)
                    var = pt4.tile([1, 512], F32, tag="var")
                    nc.vector.scalar_tensor_tensor(var, pstat[32:33, :], 1.0 / H, mu2,
                                                   op0=OP.mult, op1=OP.subtract)
                    nc.scalar.activation(var, var, AF.Sqrt, bias=eps128[0:1, :])
                    rstd = pt4.tile([1, 512], F32, tag="rstd")
                    nc.vector.reciprocal_approx_fast(out=rstd, in_=var)
                    arow = pt4.tile([1, 512], F32, tag="arow")
                    nc.vector.tensor_tensor(arow, rstd, s1row[0:1, sl], OP.mult)
                    arow_bf = pt4.tile([1, 512], BF16, tag="arow_bf")
                    nc.vector.tensor_copy(arow_bf, arow)
                    crow_bf = pt4.tile([1, 512], BF16, tag="crow_bf")
                    nc.vector.tensor_tensor(crow_bf, mu, arow, OP.mult)
                    psa = pDps.tile([P, 512], F32, tag="acc0", bufs=1, name="psa")
                    nc.tensor.matmul(psa, ones_row, arow_bf, start=True, stop=True)
                    psc = pDps.tile([P, 512], F32, tag="acc1", bufs=1, name="psc")
                    nc.tensor.matmul(psc, ones_row, crow_bf, start=True, stop=True)
                    for mc in range(HC):
                        y = xTf[:, mc, sl]
                        nc.vector.tensor_tensor(y, y, psa, OP.mult)
                        nc.vector.tensor_tensor(y, y, psc, OP.subtract)
                        if f["ln1w"]:
                            nc.vector.tensor_scalar(y, y, ln1w_sb[:, mc:mc + 1], None,
                                                    op0=OP.mult)
                        if f["ln1b"]:
                            bs = pt4.tile([P, 512], F32, tag="bs")
                            nc.vector.tensor_scalar(bs, s1_bc[:, sl],
                                                    ln1b_sb[:, mc:mc + 1],
                                                    None, op0=OP.mult)
                            nc.vector.tensor_tensor(y, y, bs, OP.add)
                        if f["beta1"]:
                            nc.vector.tensor_scalar(y, y, sct["beta1"], None,
                                                    op0=OP.add)
                        nc.scalar.activation(x1q[:, mc // 2, mc % 2, sl], y,
                                             AF.Identity, scale=YS)

                def gates(qt):
                    sl = slice(qt * 512, (qt + 1) * 512)
                    for mc in range(GC):
                        wt = pw.tile([P, DC, 2, P], FP8, tag="wproj", bufs=3)
                        nc.sync.dma_start(out=wt, in_=g1[:, :, :, mc * P:(mc + 1) * P])
                        ps = wtile([P, 512])
                        for dcp in range(DC):
                            nc.tensor.matmul(ps, wt[:, dcp, :, :],
                                             x1q[:, dcp, :, sl],
                                             start=(dcp == 0), stop=(dcp == DC - 1),
                                             perf_mode=DRM)
                        nc.scalar.activation(rT[:, mc // 2, mc % 2, sl], ps, AF.Relu,
                                             bias=gb1s_sb[:, mc:mc + 1], scale=c_g1)
                    for mc in range(HC):
                        wt = pw.tile([P, HG // 256, 2, P], FP8, tag="wg2", bufs=3)
                        nc.sync.dma_start(out=wt, in_=g2[:, :, :, mc * P:(mc + 1) * P])
                        ps = wtile([P, 512])
                        for rp in range(HG // 256):
                            nc.tensor.matmul(ps, wt[:, rp, :, :], rT[:, rp, :, sl],
                                             start=(rp == 0),
                                             stop=(rp == HG // 256 - 1),
                                             perf_mode=DRM)
                        nc.scalar.activation(gT[:, mc, :], ps, AF.Sigmoid,
                                             bias=gb2_sb[:, mc:mc + 1], scale=c_g2)

                def ffn(qt, half, hH):
                    sl = slice(qt * 512, (qt + 1) * 512)
                    for c in range(16):
                        cg = half * 16 + c
                        wt = pw.tile([P, DC, 2, P], FP8, tag="wproj", bufs=3)
                        nc.sync.dma_start(out=wt, in_=w1[:, :, :, cg * P:(cg + 1) * P])
                        psh = wtile([P, 512])
                        for dcp in range(DC):
                            nc.tensor.matmul(psh, wt[:, dcp, :, :],
                                             x1q[:, dcp, :, sl],
                                             start=(dcp == 0), stop=(dcp == DC - 1),
                                             perf_mode=DRM)
                        nc.scalar.activation(hH[:, c // 2, c % 2, sl], psh, AF.Gelu,
                                             bias=b1_sb[:, cg:cg + 1], scale=c_f1)
                    for oh in range(2):
                        acc_t = []
                        for mc in range(4):
                            acc_t.append(pDps.tile([P, 512], F32, tag=f"acc{mc}",
                                                   bufs=1, name=f"acc{mc}"))
                        for j in range(8):
                            jg = half * 8 + j
                            wt = pw.tile([P, 2, 512], FP8, tag="w2", bufs=4)
                            nc.sync.dma_start(
                                out=wt, in_=w2[:, jg, :, oh * 512:(oh + 1) * 512])
                            for mc in range(4):
                                nc.tensor.matmul(acc_t[mc],
                                                 wt[:, :, mc * P:(mc + 1) * P],
                                                 hH[:, j, :, sl],
                                                 start=(j == 0), stop=(j == 7),
                                                 perf_mode=DRM)
                        for mc in range(4):
                            mcg = oh * 4 + mc
                            a = accf[:, mcg, sl]
                            psl = acc_t[mc]
                            y = xTf[:, mcg, sl]
                            if half == 0:
                                nc.vector.scalar_tensor_tensor(
                                    a, psl, c_f2, y, op0=OP.mult, op1=OP.subtract)
                            else:
                                nc.vector.scalar_tensor_tensor(
                                    a, psl, c_f2, a, op0=OP.mult, op1=OP.add)
                                if f["b2"]:
                                    nc.vector.tensor_scalar(
                                        a, a, b2_sb[:, mcg:mcg + 1], None, op0=OP.add)
                                g = gT[:, mcg, :]
                                nc.vector.tensor_tensor(a, a, g, OP.mult)
                                nc.vector.scalar_tensor_tensor(
                                    a, y, 2.0, a, op0=OP.mult, op1=OP.add)
                                for qi in range(4):
                                    qcc = qt * 4 + qi
                                    pst = wtile([P, P])
                                    nc.tensor.transpose(
                                        pst, accf[:, mcg, qcc * P:(qcc + 1) * P],
                                        identf)
                                    nc.scalar.activation(
                                        xt2[:, qi, mcg * P:(mcg + 1) * P], pst,
                                        AF.Identity)

                def ln2_out(qt):
                    for qi in range(4):
                        qc = qt * 4 + qi
                        xt = xt2[:, qi, :]
                        stats = pt7.tile([P, 2, nc.vector.BN_STATS_DIM], F32,
                                         tag="stats")
                        for sg in range(2):
                            nc.vector.bn_stats(stats[:, sg, :],
                                               xt[:, sg * 512:(sg + 1) * 512])
                        mv = pt7.tile([P, nc.vector.BN_AGGR_DIM], F32, tag="mv")
                        nc.vector.bn_aggr(mv, stats)
                        sd = pt7.tile([P, 1], F32, tag="sd")
                        nc.scalar.activation(sd, mv[:, 1:2], AF.Sqrt, bias=eps128)
                        rstd2 = pt7.tile([P, 1], F32, tag="rstd2")
                        nc.vector.reciprocal(rstd2, sd)
                        a2 = pt7.tile([P, 1], F32, tag="a2")
                        nc.vector.tensor_tensor(a2, rstd2, s2_np[:, qc:qc + 1],
                                                OP.mult)
                        ot = pt7.tile([P, H], F32, tag="ot", bufs=1)
                        nc.vector.tensor_scalar(ot, xt, mv[:, 0:1], a2,
                                                op0=OP.subtract, op1=OP.mult)
                        if f["ln2w"]:
                            nc.vector.tensor_tensor(ot, ot, ln2w_bc, OP.mult)
                        if f["ln2b"]:
                            bs2 = pt7.tile([P, H], F32, tag="bs2")
                            nc.vector.tensor_scalar(bs2, ln2b_bc, s2_np[:, qc:qc + 1],
                                                    None, op0=OP.mult)
                            nc.vector.tensor_tensor(ot, ot, bs2, OP.add)
                        if f["beta2"]:
                            nc.vector.tensor_scalar(ot, ot, sct["beta2"], None,
                                                    op0=OP.add)
                        nc.sync.dma_start(out=out[qc * P:(qc + 1) * P, :], in_=ot)

                def halves(qt):
                    for half in range(2):
                        if half == 0:
                            hH = trunk.tile([P, 8, 2, S], FP8, tag="q8b",
                                            name="hA")
                        else:
                            hH = hB
                        ffn(qt, half, hH)

                wo_half(0)
                wo_half(1)
                ln1(0)
                gates(0)
                ln1(1)
                halves(0)
                ln2_out(0)
                gates(1)
                halves(1)
                ln2_out(1)

    nc.compile()
    return nc


def _pow2_scale(arr):
    am = float(np.max(np.abs(arr)))
    if am <= 0:
        return 1.0
    return float(2.0 ** np.floor(np.log2(200.0 / am)))


def _quant_dr(w, scale):
    f8 = ml_dtypes.float8_e4m3fn
    K, M = w.shape
    q = np.clip(w * scale, -240.0, 240.0)
    q = q.reshape(K // 256, 2, P, M).transpose(2, 0, 1, 3)
    return np.ascontiguousarray(q.astype(f8))


def _prep(inputs):
    x = np.asarray(inputs["x"], np.float32)
    volat = np.asarray(inputs["volatility"], np.float32)

    raw = {}
    for name, key in (("wq", "Wq"), ("wk", "Wk"), ("wv", "Wv"), ("wo", "Wo"),
                      ("w1", "ffn_w1"), ("w2", "ffn_w2"),
                      ("g1", "gate_w1"), ("g2", "gate_w2")):
        raw[name] = np.asarray(inputs[key], np.float32)
    ws = {name: _pow2_scale(w) for name, w in raw.items()}
    shared = {name: _quant_dr(w, ws[name]) for name, w in raw.items()}

    bq = np.asarray(inputs["bq"], np.float32)
    bk = np.asarray(inputs["bk"], np.float32)
    bv = np.asarray(inputs["bv"], np.float32)
    shared["bqs"] = np.ascontiguousarray(bq * QAS)
    shared["bks"] = np.ascontiguousarray(bk * QAS)
    shared["bvs"] = np.ascontiguousarray(bv * VS)
    gb1 = np.asarray(inputs["gate_b1"], np.float32)
    shared["gb1s"] = np.ascontiguousarray(gb1 * RS)
    for name, key in (("bo", "bo"), ("b1", "ffn_b1"), ("b2", "ffn_b2"),
                      ("gb2", "gate_b2"),
                      ("ln1w", "ln1_w"), ("ln1b", "ln1_b"),
                      ("ln2w", "ln2_w"), ("ln2b", "ln2_b")):
        shared[name] = np.ascontiguousarray(np.asarray(inputs[key], np.float32))
    for name, key in (("gamma1", "gamma1"), ("beta1", "beta1"),
                      ("vs1w", "vs1_w"), ("vs1b", "vs1_b"),
                      ("gamma2", "gamma2"), ("beta2", "beta2"),
                      ("vs2w", "vs2_w"), ("vs2b", "vs2_b")):
        shared[name] = np.asarray(inputs[key], np.float32).reshape(1)

    flags = (
        ("bv", bool(np.any(bv))),
        ("bo", bool(np.any(shared["bo"]))),
        ("b2", bool(np.any(shared["b2"]))),
        ("ln1w", bool(np.any(shared["ln1w"] != 1.0))),
        ("ln1b", bool(np.any(shared["ln1b"]))),
        ("beta1", bool(shared["beta1"][0] != 0.0)),
        ("ln2w", bool(np.any(shared["ln2w"] != 1.0))),
        ("ln2b", bool(np.any(shared["ln2b"]))),
        ("beta2", bool(shared["beta2"][0] != 0.0)),
    )
    wskey = tuple(sorted(ws.items()))

    in_maps = []
    for b in range(B):
        m = dict(shared)
        m["x"] = np.ascontiguousarray(x[b])
        m["vol"] = np.ascontiguousarray(volat[b])
        in_maps.append(m)
    return in_maps, (flags, wskey)


def _run(inputs, trace=False):
    in_maps, key = _prep(inputs)
    if key not in _BUILD_CACHE:
        _BUILD_CACHE[key] = _build(key)
    nc = _BUILD_CACHE[key]
    res = run_bass_kernel_spmd(nc, in_maps, core_ids=list(range(B)), trace=trace)
    outs = np.stack([res.results[b]["out"] for b in range(B)], axis=0)
    return outs.astype(np.float32), res


def kernel(**inputs) -> np.ndarray:
    out, _ = _run(inputs, trace=False)
    return out
